# revision 1
# baseline (speedup 1.0000x reference)
"""Trainium2 Bass kernel for nn_CrossAttentionBlock (B=4, C=512, H=W=64).

Decomposition across 8 NeuronCores: core = (batch b, query-half h).
Each core:
  stage 1: theta/phi = conv1x1(x1) packed as one [128-out] projection (PE)
  stage 2: g^T = conv1x1(x0) in [m, 64] layout + ones column (PE)
  main:    fT[m, n] = theta^T phi (PE, keys on partitions), p = exp(fT) (ACT),
           yT_ext = [g, 1]^T p accumulated over key chunks (PE) -> softmax
           numerator rows 0..63 and denominator row 64 in one accumulation.
  gather:  transpose yT -> y rows, normalize by denominator, + g_b,
           pair-wise AllGather assembles the full y for the batch.
  phase 2: W_y = W [view of y] consumed only as per-channel bn stats (AdaIN
           needs only mean/var of W_y); x0 instance stats; final out =
           r * x0 + t with per-channel scalars.

SPMD uniformity: the key/spatial axis m and the channel axis c are dummy
(contraction/stat) indices, so each core receives inputs permuted so that
"its" queries and "its" output channels come first; the host un-permutes
the output columns.
"""
import numpy as np
from contextlib import ExitStack

import concourse.bass as bass
import concourse.tile as tile
from concourse import mybir
from concourse.bass_utils import run_bass_kernel_spmd

FP32 = mybir.dt.float32
ALU = mybir.AluOpType
ACTF = mybir.ActivationFunctionType

B, C, H, W = 4, 512, 64, 64
N = H * W          # 4096 tokens
C8 = C // 8        # 64 inner channels
NH = N // 2        # 2048 queries per core
OC = C // 2        # 256 output channels per core
EPS = 1e-5

REPLICA_PAIRS = [[0, 1], [2, 3], [4, 5], [6, 7]]


def _split_excess_waits(nc, max_waits=1, drain_max=1):
    """walrus here rejects instructions carrying more than ~2 sync waits; move
    extras to preceding NoOps on the same engine (semantics preserved: waits
    run before the instruction, engine streams are sequential)."""
    for blk in nc.main_func.blocks:
        insts = blk.instructions
        k = 0
        while k < len(insts):
            inst = insts[k]
            si = inst.sync_info
            cap = drain_max if inst.opcode == "Drain" else max_waits
            if si is not None and si.on_wait and len(si.on_wait) > cap:
                waits = list(si.on_wait)
                keep = waits[-cap:]
                extra = waits[:-cap]
                pos = k
                for j in range(0, len(extra), cap):
                    nop = mybir.InstNoOp(name=f"{inst.name}-wsplit{j}", ins=[], outs=[])
                    nop.engine = inst.engine
                    nop.sync_info = mybir.SyncInfo(
                        on_wait=extra[j : j + cap], on_update=[]
                    )
                    insts.insert(pos, nop)
                    pos += 1
                    k += 1
                inst.sync_info = mybir.SyncInfo(on_wait=keep, on_update=list(si.on_update))
            k += 1


def build_nc():
    nc = bass.Bass()

    x0 = nc.dram_tensor("x0", [C, N], FP32, kind="ExternalInput")
    x1 = nc.dram_tensor("x1", [C, N], FP32, kind="ExternalInput")
    tp_wT = nc.dram_tensor("tp_wT", [C, 128], FP32, kind="ExternalInput")
    tp_b = nc.dram_tensor("tp_b", [128, 1], FP32, kind="ExternalInput")
    g_wT = nc.dram_tensor("g_wT", [C, C8], FP32, kind="ExternalInput")
    g_b_bc = nc.dram_tensor("g_b_bc", [128, C8], FP32, kind="ExternalInput")
    W_wTh = nc.dram_tensor("W_wTh", [C8, OC], FP32, kind="ExternalInput")
    W_bh = nc.dram_tensor("W_bh", [128, 2], FP32, kind="ExternalInput")
    ident = nc.dram_tensor("ident", [C8 + 1, C8 + 1], FP32, kind="ExternalInput")
    out = nc.dram_tensor("out", [OC, N], FP32, kind="ExternalOutput")

    y_bounce = nc.dram_tensor("y_bounce", [NH, C8], FP32)
    y_full = nc.dram_tensor("y_full", [N, C8], FP32)

    with tile.TileContext(nc) as tc, ExitStack() as ctx:
        wpool = ctx.enter_context(tc.tile_pool(name="weights", bufs=1))
        big = ctx.enter_context(tc.tile_pool(name="big", bufs=1))

        # ---- weights to SBUF ----
        tp_w_sb = wpool.tile([128, 4, 128], FP32)
        g_w_sb = wpool.tile([128, 4, C8], FP32)
        for c in range(4):
            nc.sync.dma_start(out=tp_w_sb[:, c, :], in_=tp_wT[c * 128:(c + 1) * 128, :])
            nc.sync.dma_start(out=g_w_sb[:, c, :], in_=g_wT[c * 128:(c + 1) * 128, :])
        tp_b_sb = wpool.tile([128, 1], FP32)
        nc.sync.dma_start(out=tp_b_sb[:], in_=tp_b[:])
        g_b_sb = wpool.tile([128, C8], FP32)
        nc.sync.dma_start(out=g_b_sb[:], in_=g_b_bc[:])
        W_w_sb = wpool.tile([C8, OC], FP32)
        nc.sync.dma_start(out=W_w_sb[:], in_=W_wTh[:])
        W_b_sb = wpool.tile([128, 2], FP32)
        nc.sync.dma_start(out=W_b_sb[:], in_=W_bh[:])
        id_sb = wpool.tile([C8 + 1, C8 + 1], FP32)
        nc.sync.dma_start(out=id_sb[:], in_=ident[:])

        # ---- persistent big tensors ----
        x0_sb = big.tile([128, 4, N], FP32)      # c-chunk on middle index
        theta_sb = big.tile([C8, N], FP32)       # keys, [64, 4096]
        phi_sb = big.tile([C8, NH], FP32)        # queries (own half), [64, 2048]
        g_extT = big.tile([128, 32, C8 + 1], FP32)  # [m-chunk, 65] per chunk
        yT_sb = big.tile([C8 + 1, NH], FP32)
        yv_sb = big.tile([C8, N], FP32)          # gathered y viewed [64, 4096]

        nc.gpsimd.memset(g_extT[:, :, C8:C8 + 1], 1.0)

        # ---- stage 1: x1 -> theta/phi ----
        with tc.tile_pool(name="x1blk", bufs=8) as x1pool, \
             tc.tile_pool(name="ps_tp", bufs=2, space="PSUM") as ps_tp:
            for blk in range(8):
                cols = slice(blk * 512, (blk + 1) * 512)
                xt = []
                for c in range(4):
                    t = x1pool.tile([128, 512], FP32)
                    nc.sync.dma_start(out=t[:], in_=x1[c * 128:(c + 1) * 128, cols])
                    xt.append(t)
                ptp = ps_tp.tile([128, 512], FP32)
                for c in range(4):
                    nc.tensor.matmul(ptp[:], tp_w_sb[:, c, :], xt[c][:],
                                     start=(c == 0), stop=(c == 3))
                nc.vector.tensor_scalar_add(theta_sb[:, cols], ptp[0:C8, :],
                                            tp_b_sb[0:C8, :])
                if blk < 4:
                    nc.vector.tensor_scalar_add(phi_sb[:, cols], ptp[C8:128, :],
                                                tp_b_sb[C8:128, :])

        # ---- stage 2: x0 -> g^T (transposed layout) ----
        with tc.tile_pool(name="ps_g", bufs=2, space="PSUM") as ps_g:
            for blk in range(8):
                cols = slice(blk * 512, (blk + 1) * 512)
                for c in range(4):
                    nc.sync.dma_start(out=x0_sb[:, c, cols],
                                      in_=x0[c * 128:(c + 1) * 128, cols])
                for mi in range(4 * blk, 4 * blk + 4):
                    pg = ps_g.tile([128, C8], FP32)
                    for c in range(4):
                        nc.tensor.matmul(pg[:],
                                         x0_sb[:, c, mi * 128:(mi + 1) * 128],
                                         g_w_sb[:, c, :],
                                         start=(c == 0), stop=(c == 3))
                    nc.vector.tensor_copy(g_extT[:, mi, 0:C8], pg[:])

        # ---- x0 instance stats (own channels = chunks 0, 1) ----
        stat = ctx.enter_context(tc.tile_pool(name="stats", bufs=1))
        x_aggs = []
        for oc in range(2):
            xst = stat.tile([128, 8, 6], FP32)
            for mb in range(8):
                nc.vector.bn_stats(xst[:, mb, :],
                                   x0_sb[:, oc, mb * 512:(mb + 1) * 512])
            xagg = stat.tile([128, 2], FP32)
            nc.vector.bn_aggr(xagg[:], xst[:])
            x_aggs.append(xagg)

        # ---- main attention loop ----
        with tc.tile_pool(name="ps_f", bufs=2, space="PSUM") as ps_f, \
             tc.tile_pool(name="ps_y", bufs=1, space="PSUM") as ps_y, \
             tc.tile_pool(name="pT", bufs=3) as ppool:
            for q in range(2):
                qc = slice(q * 1024, (q + 1) * 1024)
                py = ps_y.tile([C8 + 1, 1024], FP32)
                for mi in range(32):
                    ft = ps_f.tile([128, 1024], FP32)
                    for s in range(2):
                        nc.tensor.matmul(
                            ft[:, s * 512:(s + 1) * 512],
                            theta_sb[:, mi * 128:(mi + 1) * 128],
                            phi_sb[:, q * 1024 + s * 512: q * 1024 + (s + 1) * 512],
                            start=True, stop=True)
                    pt = ppool.tile([128, 1024], FP32)
                    nc.scalar.activation(pt[:], ft[:], ACTF.Exp)
                    for s in range(2):
                        nc.tensor.matmul(
                            py[:, s * 512:(s + 1) * 512],
                            g_extT[:, mi, :],
                            pt[:, s * 512:(s + 1) * 512],
                            start=(mi == 0), stop=(mi == 31))
                nc.vector.tensor_copy(yT_sb[:, qc], py[:])

        # ---- transpose, normalize, exchange ----
        with tc.tile_pool(name="ps_t", bufs=2, space="PSUM") as ps_t, \
             tc.tile_pool(name="ystage", bufs=3) as ystage:
            for j in range(16):
                ptile = ps_t.tile([128, C8 + 1], FP32)
                nc.tensor.transpose(ptile[:], yT_sb[:, j * 128:(j + 1) * 128], id_sb[:])
                rec = ystage.tile([128, 1], FP32, tag="rec")
                nc.vector.reciprocal(rec[:], ptile[:, C8:C8 + 1])
                yst = ystage.tile([128, C8], FP32, tag="yst")
                nc.vector.tensor_scalar_mul(yst[:], ptile[:, 0:C8], rec[:])
                nc.vector.tensor_add(yst[:], yst[:], g_b_sb[:])
                nc.sync.dma_start(out=y_bounce[j * 128:(j + 1) * 128, :], in_=yst[:])

        nc.gpsimd.collective_compute(
            "AllGather", ALU.bypass,
            replica_groups=REPLICA_PAIRS,
            ins=[y_bounce[:]],
            outs=[y_full[:]],
        )
        nc.sync.dma_start(out=yv_sb[:],
                          in_=y_full[:].rearrange("(a b) w -> a (b w)", a=C8))

        # ---- phase 2: W_y stats + per-channel affine + output ----
        with tc.tile_pool(name="ps_W", bufs=2, space="PSUM") as ps_W, \
             tc.tile_pool(name="sc", bufs=1) as sc, \
             tc.tile_pool(name="outp", bufs=2) as outp:
            for oc in range(2):
                wst = sc.tile([128, 8, 6], FP32, tag=f"wst{oc}")
                for mb in range(8):
                    pw = ps_W.tile([128, 512], FP32)
                    nc.tensor.matmul(pw[:], W_w_sb[:, oc * 128:(oc + 1) * 128],
                                     yv_sb[:, mb * 512:(mb + 1) * 512],
                                     start=True, stop=True)
                    nc.vector.bn_stats(wst[:, mb, :], pw[:])
                wagg = sc.tile([128, 2], FP32, tag=f"wagg{oc}")
                nc.vector.bn_aggr(wagg[:], wst[:])

                # r = sqrt((var_s + eps) / (var_c + eps)); t = mu_s - r*mu_c
                vc = sc.tile([128, 1], FP32, tag=f"vc{oc}")
                nc.vector.tensor_scalar_add(vc[:], x_aggs[oc][:, 1:2], EPS)
                rc = sc.tile([128, 1], FP32, tag=f"rc{oc}")
                nc.vector.reciprocal(rc[:], vc[:])
                vs = sc.tile([128, 1], FP32, tag=f"vs{oc}")
                nc.vector.tensor_scalar_add(vs[:], wagg[:, 1:2], EPS)
                ratio = sc.tile([128, 1], FP32, tag=f"ratio{oc}")
                nc.vector.tensor_mul(ratio[:], vs[:], rc[:])
                rr = sc.tile([128, 1], FP32, tag=f"rr{oc}")
                nc.scalar.sqrt(rr[:], ratio[:])
                mus = sc.tile([128, 1], FP32, tag=f"mus{oc}")
                nc.vector.tensor_add(mus[:], wagg[:, 0:1], W_b_sb[:, oc:oc + 1])
                rmc = sc.tile([128, 1], FP32, tag=f"rmc{oc}")
                nc.vector.tensor_mul(rmc[:], rr[:], x_aggs[oc][:, 0:1])
                tt = sc.tile([128, 1], FP32, tag=f"tt{oc}")
                nc.vector.tensor_sub(tt[:], mus[:], rmc[:])

                for mb in range(4):
                    cols = slice(mb * 1024, (mb + 1) * 1024)
                    ot = outp.tile([128, 1024], FP32)
                    nc.vector.tensor_scalar(ot[:], x0_sb[:, oc, cols], rr[:], tt[:],
                                            ALU.mult, ALU.add)
                    nc.sync.dma_start(out=out[oc * 128:(oc + 1) * 128, cols], in_=ot[:])

    _split_excess_waits(nc)
    return nc


_NC_CACHE = None


def _get_nc():
    global _NC_CACHE
    if _NC_CACHE is None:
        _NC_CACHE = build_nc()
    return _NC_CACHE


def _core_inputs(x0f, x1f, tp_wT, tp_b, g_wT, g_b, W_wT, W_b, ident, core):
    b, half = core // 2, core % 2
    x0b, x1b = x0f[b], x1f[b]
    if half == 0:
        x0p = x0b
        x1p = x1b
        g_wp = g_wT
    else:
        # queries-first column permutation; own-channels-first row permutation
        x1p = np.concatenate([x1b[:, NH:], x1b[:, :NH]], axis=1)
        x0r = np.concatenate([x0b[OC:], x0b[:OC]], axis=0)
        x0p = np.concatenate([x0r[:, NH:], x0r[:, :NH]], axis=1)
        g_wp = np.concatenate([g_wT[OC:], g_wT[:OC]], axis=0)
    return {
        "x0": np.ascontiguousarray(x0p),
        "x1": np.ascontiguousarray(x1p),
        "tp_wT": tp_wT,
        "tp_b": tp_b,
        "g_wT": np.ascontiguousarray(g_wp),
        "g_b_bc": np.ascontiguousarray(np.broadcast_to(g_b, (128, C8))),
        "W_wTh": np.ascontiguousarray(W_wT[:, half * OC:(half + 1) * OC]),
        "W_bh": np.ascontiguousarray(
            W_b[half * OC:(half + 1) * OC].reshape(2, 128).T),
        "ident": ident,
    }


def kernel(x0, x1, g_w, g_b, theta_w, theta_b, phi_w, phi_b, W_w, W_b):
    x0 = np.asarray(x0, dtype=np.float32)
    x1 = np.asarray(x1, dtype=np.float32)
    x0f = x0.reshape(B, C, N)
    x1f = x1.reshape(B, C, N)
    tp_wT = np.ascontiguousarray(
        np.concatenate([theta_w, phi_w], axis=0).T.astype(np.float32))
    tp_b = np.ascontiguousarray(
        np.concatenate([theta_b, phi_b]).astype(np.float32)[:, None])
    g_wT = np.ascontiguousarray(np.asarray(g_w, np.float32).T)
    W_wT = np.ascontiguousarray(np.asarray(W_w, np.float32).T)
    ident = np.eye(C8 + 1, dtype=np.float32)
    g_b = np.asarray(g_b, np.float32)
    W_b = np.asarray(W_b, np.float32)

    in_maps = [
        _core_inputs(x0f, x1f, tp_wT, tp_b, g_wT, g_b, W_wT, W_b, ident, core)
        for core in range(8)
    ]
    nc = _get_nc()
    res = run_bass_kernel_spmd(nc, in_maps, core_ids=list(range(8)))

    out = np.empty((B, C, N), dtype=np.float32)
    for core in range(8):
        b, half = core // 2, core % 2
        o = res.results[core]["out"]
        if half == 1:
            o = np.concatenate([o[:, NH:], o[:, :NH]], axis=1)
        out[b, half * OC:(half + 1) * OC] = o
    return out.reshape(B, C, H, W)



# revision 13
# speedup vs baseline: 2.3245x; 2.3245x over previous
"""Trainium2 Bass kernel for nn_CrossAttentionBlock (B=4, C=512, H=W=64).

Decomposition across 8 NeuronCores: core = (batch b, spatial half h), where
the query/token split is INTERLEAVED: core h owns tokens {t : 32h <= t%64 < 32h+32}.
With the faithful torch-.view semantics (y [B,N,C8] reinterpreted as
[B,C8,H,W]), view-channel c at view-spatial (h',w) reads y[token c*64+h', w],
so a core owning all tokens with t%64 in its half can reconstruct the FULL
contraction input yv[:, j] for its half of the view-spatial axis locally --
no exchange of y is needed. AdaIN consumes W_y only through per-channel
spatial statistics, which are additive over the spatial axis, so the cores
in a pair exchange tiny partial bn-statistics (x0 stats and W_y stats)
instead of the y tensor.

All matmuls run in bf16 (inputs host-cast / cast on write), fp32 PSUM
accumulation; final output math in fp32. Measured end-to-end error vs the
fp32 reference ~1e-3 max-rel, well inside the 2e-2 gate.

Per core:
  stage 1 (interleaved with main loop): theta/phi = conv1x1(x1p) packed as
           one 128-out projection.
  stage 2 (interleaved): g^T via x0h-as-stationary trick -> [token, ch].
  main:    fT[key,1024q] = theta^T phi (PE), p = exp(fT) (ACT, ->bf16),
           yT_ext accumulated over 32 key chunks (PE).
  post-q:  PE-transpose yT chunks, normalize by denominator row, bounce
           through DRAM to build yv [viewch, 2048] locally.
  stats:   x0 partial stats (during main loop) + W_y partial stats (tail),
           each exchanged pairwise via small AllGather.
  tail:    combine stats, per-channel affine out = r*x0 + t.
"""
import numpy as np
import ml_dtypes
from contextlib import ExitStack

import concourse.bass as bass
import concourse.tile as tile
from concourse import mybir
from concourse.bass_utils import run_bass_kernel_spmd

FP32 = mybir.dt.float32
BF16 = mybir.dt.bfloat16
ALU = mybir.AluOpType
ACTF = mybir.ActivationFunctionType

B, C, H, W = 4, 512, 64, 64
N = H * W          # 4096 tokens
C8 = C // 8        # 64 inner channels
NH = N // 2        # 2048 tokens per core
HC = NH            # spatial half size (columns of out per core)
EPS = 1e-5

REPLICA_PAIRS = [[0, 1], [2, 3], [4, 5], [6, 7]]

_T0 = np.array([c * 64 + j for c in range(64) for j in range(32)])
_T1 = _T0 + 32
PERMS = [np.concatenate([_T0, _T1]), np.concatenate([_T1, _T0])]

NPBF16 = ml_dtypes.bfloat16


def _split_excess_waits(nc, max_waits=1, drain_max=1):
    """walrus rejects instructions carrying more than ~2 sync waits; move
    extras to preceding NoOps on the same engine (semantics preserved: waits
    run before the instruction, engine streams are sequential)."""
    for blk in nc.main_func.blocks:
        insts = blk.instructions
        k = 0
        while k < len(insts):
            inst = insts[k]
            si = inst.sync_info
            cap = drain_max if inst.opcode == "Drain" else max_waits
            if si is not None and si.on_wait and len(si.on_wait) > cap:
                waits = list(si.on_wait)
                keep = waits[-cap:]
                extra = waits[:-cap]
                pos = k
                for j in range(0, len(extra), cap):
                    nop = mybir.InstNoOp(name=f"{inst.name}-wsplit{j}", ins=[], outs=[])
                    nop.engine = inst.engine
                    nop.sync_info = mybir.SyncInfo(
                        on_wait=extra[j : j + cap], on_update=[]
                    )
                    insts.insert(pos, nop)
                    pos += 1
                    k += 1
                inst.sync_info = mybir.SyncInfo(on_wait=keep, on_update=list(si.on_update))
            k += 1


def build_nc():
    nc = bass.Bass()

    x1t = nc.dram_tensor("x1t", [128, 8, 4, 512], BF16, kind="ExternalInput")
    x0ht = nc.dram_tensor("x0ht", [128, 8, 4, 512], BF16, kind="ExternalInput")
    x0at = nc.dram_tensor("x0at", [128, 4, HC], FP32, kind="ExternalInput")
    tp_wT = nc.dram_tensor("tp_wT", [C, 128], BF16, kind="ExternalInput")
    tp_b = nc.dram_tensor("tp_b", [128, 1], FP32, kind="ExternalInput")
    g_wT = nc.dram_tensor("g_wT", [C, C8], BF16, kind="ExternalInput")
    g_b_bc = nc.dram_tensor("g_b_bc", [128, C8], FP32, kind="ExternalInput")
    W_wT = nc.dram_tensor("W_wT", [C8, C], BF16, kind="ExternalInput")
    W_b4 = nc.dram_tensor("W_b4", [128, 4], FP32, kind="ExternalInput")
    ident = nc.dram_tensor("ident", [C8 + 1, C8 + 1], FP32, kind="ExternalInput")
    out = nc.dram_tensor("out", [C, HC], FP32, kind="ExternalOutput")

    y_loc = nc.dram_tensor("y_loc", [NH, C8], BF16)
    # partial bn-stats exchange buffers: row p holds that partition's
    # [oc(4) x mb(4) x 6] stats flattened; AllGather stacks partner at 128..255
    xst_send = nc.dram_tensor("xst_send", [128, 96], FP32)
    xst_recv = nc.dram_tensor("xst_recv", [256, 96], FP32)
    wst_send = nc.dram_tensor("wst_send", [128, 96], FP32)
    wst_recv = nc.dram_tensor("wst_recv", [256, 96], FP32)

    with tile.TileContext(nc) as tc, ExitStack() as ctx:
        wpool = ctx.enter_context(tc.tile_pool(name="weights", bufs=1))
        big = ctx.enter_context(tc.tile_pool(name="big", bufs=1))

        # ---- weights to SBUF ----
        tp_w_sb = wpool.tile([128, 4, 128], BF16)
        nc.sync.dma_start(out=tp_w_sb[:], in_=tp_wT[:].rearrange("(c p) o -> p c o", c=4))
        g_w_sb = wpool.tile([128, 4, C8], BF16)
        nc.sync.dma_start(out=g_w_sb[:], in_=g_wT[:].rearrange("(c p) o -> p c o", c=4))
        tp_b_sb = wpool.tile([128, 1], FP32)
        nc.sync.dma_start(out=tp_b_sb[:], in_=tp_b[:])
        g_b_sb = wpool.tile([128, C8], FP32)
        nc.sync.dma_start(out=g_b_sb[:], in_=g_b_bc[:])
        W_w_sb = wpool.tile([C8, C], BF16)
        nc.sync.dma_start(out=W_w_sb[:], in_=W_wT[:])
        W_b_sb = wpool.tile([128, 4], FP32)
        nc.sync.dma_start(out=W_b_sb[:], in_=W_b4[:])
        id_sb = wpool.tile([C8 + 1, C8 + 1], FP32)
        nc.sync.dma_start(out=id_sb[:], in_=ident[:])

        # ---- persistent big tensors ----
        theta_sb = big.tile([C8, N], BF16)        # keys [64, 4096]
        phi_sb = big.tile([C8, NH], BF16)         # own queries [64, 2048]
        g_extT = big.tile([128, 32, C8 + 1], BF16)  # [key-chunk, mi, 65]
        yT_sb = big.tile([C8 + 1, NH], FP32)
        yv_sb = big.tile([C8, HC], BF16)          # local view input [64, 2048]
        x0a_sb = big.tile([128, 4, HC], FP32)     # x0 own spatial half
        xs_st = big.tile([128, 4, 4, 6], FP32)    # x0 partial bn stats
        ws_st = big.tile([128, 4, 4, 6], FP32)    # W_y partial bn stats
        xall_sb = big.tile([128, 2, 4, 4, 6], FP32)   # [p, replica, oc, mb, 6]
        wall_sb = big.tile([128, 2, 4, 4, 6], FP32)

        nc.gpsimd.memset(g_extT[:, :, C8:C8 + 1], 1.0)

        # main PSUM pool: ft (2 bufs x 2 banks) + py (1 buf x 2 banks) = 6 banks
        ps_main = ctx.enter_context(tc.tile_pool(name="ps_main", bufs=2, space="PSUM"))
        ppool = ctx.enter_context(tc.tile_pool(name="pT", bufs=3))

        stage_ctx = ExitStack()
        x1pool = stage_ctx.enter_context(tc.tile_pool(name="x1blk", bufs=3))
        x0pool = stage_ctx.enter_context(tc.tile_pool(name="x0blk", bufs=3))
        ps_st = stage_ctx.enter_context(tc.tile_pool(name="ps_st", bufs=2, space="PSUM"))

        def stage_blk(blk):
            x1b = x1pool.tile([128, 4, 512], BF16, tag="x1")
            nc.sync.dma_start(out=x1b[:], in_=x1t[:, blk, :, :])
            x0b = x0pool.tile([128, 4, 512], BF16, tag="x0")
            nc.sync.dma_start(out=x0b[:], in_=x0ht[:, blk, :, :])
            ptp = ps_st.tile([128, 512], FP32, tag="ps")
            for c in range(4):
                nc.tensor.matmul(ptp[:], tp_w_sb[:, c, :], x1b[:, c, :],
                                 start=(c == 0), stop=(c == 3))
            cols = slice(blk * 512, (blk + 1) * 512)
            nc.vector.tensor_scalar_add(theta_sb[:, cols], ptp[0:C8, :],
                                        tp_b_sb[0:C8, :])
            if blk < 4:
                nc.vector.tensor_scalar_add(phi_sb[:, cols], ptp[C8:128, :],
                                            tp_b_sb[C8:128, :])
            for k in range(4):
                mi = blk * 4 + k
                pg = ps_st.tile([128, 512], FP32, tag="ps")
                for c in range(4):
                    nc.tensor.matmul(pg[:, 0:C8], x0b[:, c, k * 128:(k + 1) * 128],
                                     g_w_sb[:, c, :], start=(c == 0), stop=(c == 3))
                nc.vector.tensor_copy(g_extT[:, mi, 0:C8], pg[:, 0:C8])

        stage_blk(0)
        stage_blk(1)

        def main_mi(q, mi, py):
            ft = ps_main.tile([128, 1024], FP32, tag="ft")
            for s in range(2):
                nc.tensor.matmul(
                    ft[:, s * 512:(s + 1) * 512],
                    theta_sb[:, mi * 128:(mi + 1) * 128],
                    phi_sb[:, q * 1024 + s * 512: q * 1024 + (s + 1) * 512],
                    start=True, stop=True)
            pt = ppool.tile([128, 1024], BF16, tag="pt")
            nc.scalar.activation(pt[:], ft[:], ACTF.Exp)
            for s in range(2):
                nc.tensor.matmul(
                    py[:, s * 512:(s + 1) * 512],
                    g_extT[:, mi, :],
                    pt[:, s * 512:(s + 1) * 512],
                    start=(mi == 0), stop=(mi == 31))

        def xpose_norm(j, ps_t, ystage):
            ptile = ps_t.tile([128, C8 + 1], FP32, tag="pt")
            nc.tensor.transpose(ptile[:], yT_sb[:, j * 128:(j + 1) * 128], id_sb[:])
            rec = ystage.tile([128, 1], FP32, tag="rec")
            nc.vector.reciprocal(rec[:], ptile[:, C8:C8 + 1])
            tmp = ystage.tile([128, C8], FP32, tag="tmp")
            nc.vector.tensor_scalar_mul(tmp[:], ptile[:, 0:C8], rec[:])
            yst = ystage.tile([128, C8], BF16, tag="yst")
            nc.vector.tensor_add(yst[:], tmp[:], g_b_sb[:])
            nc.sync.dma_start(out=y_loc[j * 128:(j + 1) * 128, :], in_=yst[:])

        # ---- q0 main loop, stages 2..7 interleaved ----
        py0 = ps_main.tile([C8 + 1, 1024], FP32, tag="py", bufs=1)
        for mi in range(32):
            if mi >= 4 and mi % 4 == 0 and mi // 4 + 1 <= 7:
                stage_blk(mi // 4 + 1)
            if mi == 13:
                # x0 fp32 own-half loads; streams during main loop
                for cch in range(4):
                    nc.sync.dma_start(out=x0a_sb[:, cch, :], in_=x0at[:, cch, :])
            main_mi(0, mi, py0)
        nc.vector.tensor_copy(yT_sb[:, 0:1024], py0[:])
        stage_ctx.close()

        t1_ctx = ExitStack()
        ps_t = t1_ctx.enter_context(tc.tile_pool(name="ps_t", bufs=2, space="PSUM"))
        ystage = t1_ctx.enter_context(tc.tile_pool(name="ystage", bufs=2))

        # ---- q1 main loop; q0 transpose/normalize + x0 stats interleaved ----
        py1 = ps_main.tile([C8 + 1, 1024], FP32, tag="py", bufs=1)
        for mi in range(32):
            main_mi(1, mi, py1)
            if mi >= 2 and mi % 2 == 0 and (mi - 2) // 2 <= 7:
                xpose_norm((mi - 2) // 2, ps_t, ystage)
            if mi == 19:
                nc.sync.dma_start(
                    out=yv_sb[0:32, :],
                    in_=y_loc[0:1024, :].rearrange("(a b) w -> a (b w)", a=32))
            if mi == 21:
                for oc in range(4):
                    for mb in range(4):
                        nc.vector.bn_stats(xs_st[:, oc, mb, :],
                                           x0a_sb[:, oc, mb * 512:(mb + 1) * 512])
                nc.sync.dma_start(out=xst_send[:], in_=xs_st[:])
                nc.gpsimd.collective_compute(
                    "AllGather", ALU.bypass,
                    replica_groups=REPLICA_PAIRS,
                    ins=[xst_send[:]],
                    outs=[xst_recv[:]],
                )
                nc.sync.dma_start(
                    out=xall_sb[:],
                    in_=xst_recv[:].rearrange("(w p) g -> p w g", w=2))
        nc.vector.tensor_copy(yT_sb[:, 1024:2048], py1[:])
        for j in range(8, 16):
            xpose_norm(j, ps_t, ystage)
        nc.sync.dma_start(
            out=yv_sb[32:64, :],
            in_=y_loc[1024:2048, :].rearrange("(a b) w -> a (b w)", a=32))
        t1_ctx.close()

        tail_ctx = ExitStack()
        ps_W = tail_ctx.enter_context(tc.tile_pool(name="ps_W", bufs=2, space="PSUM"))
        sc = tail_ctx.enter_context(tc.tile_pool(name="sc", bufs=1))
        outp = tail_ctx.enter_context(tc.tile_pool(name="outp", bufs=2))
        for oc in range(4):
            for mb in range(4):
                pw = ps_W.tile([128, 512], FP32, tag="pw")
                nc.tensor.matmul(pw[:], W_w_sb[:, oc * 128:(oc + 1) * 128],
                                 yv_sb[:, mb * 512:(mb + 1) * 512],
                                 start=True, stop=True)
                nc.vector.bn_stats(ws_st[:, oc, mb, :], pw[:])
        nc.sync.dma_start(out=wst_send[:], in_=ws_st[:])
        nc.gpsimd.collective_compute(
            "AllGather", ALU.bypass,
            replica_groups=REPLICA_PAIRS,
            ins=[wst_send[:]],
            outs=[wst_recv[:]],
        )
        nc.sync.dma_start(
            out=wall_sb[:],
            in_=wst_recv[:].rearrange("(w p) g -> p w g", w=2))

        # ---- combine stats, per-channel affine ----
        xagg = sc.tile([128, 4, 2], FP32, tag="xagg")
        wagg = sc.tile([128, 4, 2], FP32, tag="wagg")
        for oc in range(4):
            nc.vector.bn_aggr(xagg[:, oc, :], xall_sb[:, :, oc])
            nc.vector.bn_aggr(wagg[:, oc, :], wall_sb[:, :, oc])

        vc = sc.tile([128, 4], FP32, tag="vc")
        nc.vector.tensor_scalar_add(vc[:], xagg[:, :, 1], EPS)
        rc = sc.tile([128, 4], FP32, tag="rc")
        nc.vector.reciprocal(rc[:], vc[:])
        vs = sc.tile([128, 4], FP32, tag="vs")
        nc.vector.tensor_scalar_add(vs[:], wagg[:, :, 1], EPS)
        ratio = sc.tile([128, 4], FP32, tag="ratio")
        nc.vector.tensor_mul(ratio[:], vs[:], rc[:])
        rr = sc.tile([128, 4], FP32, tag="rr")
        nc.scalar.sqrt(rr[:], ratio[:])
        mus = sc.tile([128, 4], FP32, tag="mus")
        nc.vector.tensor_add(mus[:], wagg[:, :, 0], W_b_sb[:])
        rmc = sc.tile([128, 4], FP32, tag="rmc")
        nc.vector.tensor_mul(rmc[:], rr[:], xagg[:, :, 0])
        tt = sc.tile([128, 4], FP32, tag="tt")
        nc.vector.tensor_sub(tt[:], mus[:], rmc[:])

        for oc in range(4):
            ot = outp.tile([128, HC], FP32, tag="ot")
            nc.vector.tensor_scalar(ot[:, 0:1024], x0a_sb[:, oc, 0:1024],
                                    rr[:, oc:oc + 1], tt[:, oc:oc + 1],
                                    ALU.mult, ALU.add)
            nc.gpsimd.tensor_scalar(ot[:, 1024:2048], x0a_sb[:, oc, 1024:2048],
                                    rr[:, oc:oc + 1], tt[:, oc:oc + 1],
                                    ALU.mult, ALU.add)
            nc.sync.dma_start(out=out[oc * 128:(oc + 1) * 128, :], in_=ot[:])

        tail_ctx.close()

    _split_excess_waits(nc)
    return nc


_NC_CACHE = None


def _get_nc():
    global _NC_CACHE
    if _NC_CACHE is None:
        _NC_CACHE = build_nc()
    return _NC_CACHE


def _prep_weights(g_w, g_b, theta_w, theta_b, phi_w, phi_b, W_w, W_b):
    tp_wT = np.ascontiguousarray(
        np.concatenate([theta_w, phi_w], axis=0).T).astype(NPBF16)
    tp_b = np.ascontiguousarray(
        np.concatenate([theta_b, phi_b]).astype(np.float32)[:, None])
    g_wT = np.ascontiguousarray(np.asarray(g_w, np.float32).T).astype(NPBF16)
    g_b_bc = np.ascontiguousarray(
        np.broadcast_to(np.asarray(g_b, np.float32), (128, C8)))
    W_wT = np.ascontiguousarray(np.asarray(W_w, np.float32).T).astype(NPBF16)
    W_b4 = np.ascontiguousarray(
        np.asarray(W_b, np.float32).reshape(4, 128).T)
    ident = np.eye(C8 + 1, dtype=np.float32)
    return tp_wT, tp_b, g_wT, g_b_bc, W_wT, W_b4, ident


def _core_inputs(x0f, x1f, weights, core):
    tp_wT, tp_b, g_wT, g_b_bc, W_wT, W_b4, ident = weights
    b, h = core // 2, core % 2
    perm = PERMS[h]
    x1p = x1f[b][:, perm]
    x0hp = x0f[b][:, perm]
    x0aff = x0f[b][:, h * HC:(h + 1) * HC]
    x1t = np.ascontiguousarray(
        x1p.reshape(4, 128, 8, 512).transpose(1, 2, 0, 3)).astype(NPBF16)
    x0ht = np.ascontiguousarray(
        x0hp.reshape(4, 128, 8, 512).transpose(1, 2, 0, 3)).astype(NPBF16)
    x0at = np.ascontiguousarray(
        x0aff.reshape(4, 128, HC).transpose(1, 0, 2))
    return {
        "x1t": x1t,
        "x0ht": x0ht,
        "x0at": x0at,
        "tp_wT": tp_wT,
        "tp_b": tp_b,
        "g_wT": g_wT,
        "g_b_bc": g_b_bc,
        "W_wT": W_wT,
        "W_b4": W_b4,
        "ident": ident,
    }


def kernel(x0, x1, g_w, g_b, theta_w, theta_b, phi_w, phi_b, W_w, W_b):
    x0 = np.asarray(x0, dtype=np.float32)
    x1 = np.asarray(x1, dtype=np.float32)
    x0f = x0.reshape(B, C, N)
    x1f = x1.reshape(B, C, N)
    weights = _prep_weights(g_w, g_b, theta_w, theta_b, phi_w, phi_b, W_w, W_b)

    in_maps = [_core_inputs(x0f, x1f, weights, core) for core in range(8)]
    nc = _get_nc()
    res = run_bass_kernel_spmd(nc, in_maps, core_ids=list(range(8)))

    out = np.empty((B, C, N), dtype=np.float32)
    for core in range(8):
        b, h = core // 2, core % 2
        out[b][:, h * HC:(h + 1) * HC] = res.results[core]["out"]
    return out.reshape(B, C, H, W)
